# revision 33
# baseline (speedup 1.0000x reference)
"""AvU loss (nn_AUAvULoss) Trainium2 kernel.

Strategy (8 NeuronCores, data-parallel over the sample axis):
  Launch 1 (tiny): per-core partial min/max of `unc` -> host reduces to
  global umin/umax (the 2-scalar all-reduce from the sharding hint).
  Launch 2 (main): per core, over its 262144-sample shard laid out as
  [128 partitions x 2048], processed in 512-wide free-dim blocks so
  DMA, mask generation and TensorEngine consumption pipeline:
    - elementwise prep on DVE/ACT produces 8 bf16 "stationary" columns
      per sample: [1, m, p1, t, m*t, p1*t, m*p1, m*p1*t]
      (m = prediction-correct mask, p1 = confidence, t = tanh(unc))
    - 21 threshold masks (ones column + 20 compare columns, split across
      DVE is_le and ACT Sign) form the bf16 "moving" columns
    - the TensorEngine contracts 128 samples per 128-column group:
      16 sample-chunks share one [128,128] stationary load and one
      matmul with a [128, 21*16] moving operand; the 16 diagonal
      [8 x 21] blocks of the PSUM output are the real per-chunk sums
      (off-diagonal chunk cross-terms are ignored at readout).
  Host combines the partial sums in float64 and evaluates the AvU
  ratios, trapezoid AUC and log loss.
"""

import numpy as np

import concourse.bass as bass
import concourse.bacc as bacc
import concourse.tile as tile
from concourse import mybir
from concourse.bass_utils import run_bass_kernel_spmd

N_TOTAL = 2_097_152
N_CORES = 8
NS = N_TOTAL // N_CORES  # 262144 samples per core
P = 128
F = NS // P              # 2048 free elements per partition
N_TH = 21
FB = 512                 # free-dim block size for the mask/stat pipeline
N_BLK = F // FB
G = 16                   # sample-chunks fused per matmul (16*8 = 128 stationary cols)
GB = FB // G             # chunk-groups per block
EPS = 1e-10
BETA = 1.0

F32 = mybir.dt.float32
I32 = mybir.dt.int32
BF16 = mybir.dt.bfloat16

# Engine per threshold mask k=0..19 (k=20 is the constant ones column):
#   'v' -> DVE tensor_scalar is_le -> {0,1}
#   'a' -> ACT Sign(th_k - unc)    -> {-1,0,1}
MASK_ENG = ['v', 'a', 'v', 'a', 'v', 'a', 'v', 'a', 'v', 'a',
            'v', 'a', 'v', 'a', 'v', 'a', 'v', 'a', 'v', 'v']

_CACHE = {}
FUSED = False
LAST_RESULTS = []  # (name, BassKernelResults) for test introspection
TRACE = False


def _build_minmax():
    """Raw-bass (no Tile) min/max: chunked DMA overlapped with reduces,
    no Tile tail barrier. Output [P, 8]: per-chunk min in cols 0..3,
    max in cols 4..7; host reduces the rest."""
    nc = bacc.Bacc("TRN2", target_bir_lowering=False, debug=False)
    unc_d = nc.dram_tensor("unc", [NS], F32, kind="ExternalInput")
    out_d = nc.dram_tensor("mm", [P, 8], F32, kind="ExternalOutput")
    unc_pa = unc_d.ap().rearrange("(p a) -> p a", p=P)
    NCK = 4
    CK = F // NCK
    with (
        nc.sbuf_tensor("unc_t", [P, F], F32) as unc_t,
        nc.sbuf_tensor("mm_t", [P, 8], F32) as mm_t,
        nc.semaphore("s_dma0") as s_dma0,
        nc.semaphore("s_dma1") as s_dma1,
        nc.semaphore("s_dma2") as s_dma2,
        nc.semaphore("s_dma3") as s_dma3,
        nc.semaphore("s_out") as s_out,
        nc.semaphore("s_v") as s_v,
        nc.Block() as block,
    ):
        s_dma = [s_dma0, s_dma1, s_dma2, s_dma3]

        @block.sync
        def _(sync):
            # one semaphore per chunk: SDMA engines complete out of order
            # across queued DMAs, so a shared counter would not imply that
            # earlier chunks have landed.
            for i in range(NCK):
                sync.dma_start(
                    out=unc_t[:, i * CK:(i + 1) * CK],
                    in_=unc_pa[:, i * CK:(i + 1) * CK],
                ).then_inc(s_dma[i], 16)
            sync.wait_ge(s_v, 2 * NCK)
            sync.dma_start(out=out_d.ap(), in_=mm_t[:]).then_inc(s_out, 16)

        @block.vector
        def _(vector):
            for i in range(NCK):
                vector.wait_ge(s_dma[i], 16)
                src = unc_t[:, i * CK:(i + 1) * CK]
                vector.tensor_reduce(out=mm_t[:, i:i + 1], in_=src,
                                     axis=mybir.AxisListType.X,
                                     op=mybir.AluOpType.min).then_inc(s_v, 1)
                vector.tensor_reduce(out=mm_t[:, 4 + i:5 + i], in_=src,
                                     axis=mybir.AxisListType.X,
                                     op=mybir.AluOpType.max).then_inc(s_v, 1)
    nc.compile()
    return nc


def _build_main():
    nc = bacc.Bacc("TRN2", target_bir_lowering=False, debug=False)
    probs_d = nc.dram_tensor("probs", [NS, 2], F32, kind="ExternalInput")
    lab_d = nc.dram_tensor("lab", [NS, 2], I32, kind="ExternalInput")
    unc_d = nc.dram_tensor("unc", [NS], F32, kind="ExternalInput")
    th_d = nc.dram_tensor("th", [P, N_TH], F32, kind="ExternalInput")
    out_d = nc.dram_tensor("out", [P, N_TH * G], F32, kind="ExternalOutput")

    probs_pa = probs_d.ap().rearrange("(p a) c -> p (a c)", p=P)  # [128, 4096]
    lab_pa = lab_d.ap().rearrange("(p a) c -> p (a c)", p=P)      # [128, 4096] i32
    unc_pa = unc_d.ap().rearrange("(p a) -> p a", p=P)            # [128, 2048]

    with tile.TileContext(nc) as tc:
        with (
            tc.tile_pool(name="consts", bufs=1) as pc,
            tc.tile_pool(name="inblk", bufs=2) as pin,
            tc.tile_pool(name="work", bufs=2) as pw,
            tc.tile_pool(name="psum", bufs=1, space="PSUM") as pps,
        ):
            th = pc.tile([P, N_TH], F32)
            nc.sync.dma_start(out=th, in_=th_d.ap())
            psum_t = pps.tile([P, N_TH * G], F32)  # 336 f32 -> one bank

            for b in range(N_BLK):
                lo2 = b * 2 * FB
                lo1 = b * FB
                unc_b = pin.tile([P, FB], F32, tag="unc")
                nc.sync.dma_start(out=unc_b, in_=unc_pa[:, lo1:lo1 + FB])
                probs_b = pin.tile([P, 2 * FB], F32, tag="probs")
                nc.sync.dma_start(out=probs_b, in_=probs_pa[:, lo2:lo2 + 2 * FB])
                lab_b = pin.tile([P, 2 * FB], I32, tag="lab")
                nc.sync.dma_start(out=lab_b, in_=lab_pa[:, lo2:lo2 + 2 * FB])

                p1v = probs_b[:, 1::2]
                p0v = probs_b[:, 0::2]

                # stat[p, g, q, jw]: group-g stationary = stat[:, g] is a
                # contiguous [128, 8*16] slab (q-major, chunk-within-group jw
                # minor). mask[p, g, kk, jw] likewise -> moving [128, 21*16].
                stat = pw.tile([P, GB, 8, G], BF16, tag="stat")
                maskt = pw.tile([P, GB, N_TH, G], BF16, tag="mask")
                labf = pw.tile([P, FB], BF16, tag="labf")
                pred = pw.tile([P, FB], BF16, tag="pred")

                def qcol(q):
                    # [P, GB, G] view of stationary column q
                    return stat[:, :, q, :]

                lv = labf.rearrange("p (g j) -> p g j", j=G)
                pv = pred.rearrange("p (g j) -> p g j", j=G)
                uv = unc_b.rearrange("p (g j) -> p g j", j=G)
                p1g = probs_b.rearrange("p (g j c) -> p g (j c)", g=GB, c=2)[:, :, 1::2]

                # per-sample quantities -> stationary columns
                nc.vector.tensor_copy(labf, lab_b[:, 0::2])
                nc.vector.tensor_tensor(out=pred, in0=p1v, in1=p0v,
                                        op=mybir.AluOpType.is_gt)
                nc.vector.tensor_tensor(out=qcol(1), in0=pv, in1=lv,
                                        op=mybir.AluOpType.is_equal)
                nc.gpsimd.memset(qcol(0), 1.0)
                nc.vector.tensor_copy(qcol(2), p1g)
                nc.scalar.activation(out=qcol(3), in_=uv,
                                     func=mybir.ActivationFunctionType.Tanh)
                nc.vector.tensor_tensor(out=qcol(4), in0=qcol(1), in1=qcol(3),
                                        op=mybir.AluOpType.mult)
                nc.vector.tensor_tensor(out=qcol(5), in0=qcol(2), in1=qcol(3),
                                        op=mybir.AluOpType.mult)
                nc.vector.tensor_tensor(out=qcol(6), in0=qcol(1), in1=qcol(2),
                                        op=mybir.AluOpType.mult)
                nc.vector.tensor_tensor(out=qcol(7), in0=qcol(6), in1=qcol(3),
                                        op=mybir.AluOpType.mult)

                # threshold masks -> moving columns
                nc.gpsimd.memset(maskt[:, :, 0, :], 1.0)
                for k, eng in enumerate(MASK_ENG):
                    dst = maskt[:, :, 1 + k, :]
                    thk = th[:, k:k + 1]
                    if eng == 'v':
                        nc.vector.tensor_scalar(out=dst, in0=uv, scalar1=thk,
                                                scalar2=None, op0=mybir.AluOpType.is_le)
                    else:
                        nc.scalar.activation(out=dst, in_=uv,
                                             func=mybir.ActivationFunctionType.Sign,
                                             bias=thk, scale=-1.0)

                # PE: one [128,128] stationary + one FD=336 matmul per group
                for g in range(GB):
                    gg = b * GB + g
                    nc.tensor.matmul(
                        out=psum_t,
                        lhsT=stat[:, g, :, :],
                        rhs=maskt[:, g, :, :],
                        start=(gg == 0),
                        stop=(gg == N_BLK * GB - 1),
                    )

            out_sb = pc.tile([P, N_TH * G], F32)
            nc.vector.tensor_copy(out_sb, psum_t)
            nc.sync.dma_start(out=out_d.ap(), in_=out_sb)
    nc.compile()
    return nc


def _build_fused():
    """Single-launch variant: umin/umax all-reduce happens on-device via
    an 8-core AllReduce that overlaps threshold-independent prep work."""
    import concourse.bass_isa as bass_isa
    nc = bacc.Bacc("TRN2", target_bir_lowering=False, debug=False, num_devices=N_CORES)
    probs_d = nc.dram_tensor("probs", [NS, 2], F32, kind="ExternalInput")
    lab_d = nc.dram_tensor("lab", [NS, 2], I32, kind="ExternalInput")
    unc_d = nc.dram_tensor("unc", [NS], F32, kind="ExternalInput")
    lin_d = nc.dram_tensor("lin", [P, N_TH], F32, kind="ExternalInput")
    out_d = nc.dram_tensor("out", [P, N_TH * G], F32, kind="ExternalOutput")

    probs_pa = probs_d.ap().rearrange("(p a) c -> p (a c)", p=P)
    lab_pa = lab_d.ap().rearrange("(p a) c -> p (a c)", p=P)
    unc_pa = unc_d.ap().rearrange("(p a) -> p a", p=P)

    with tile.TileContext(nc) as tc:
        with (
            tc.tile_pool(name="consts", bufs=1) as pc,
            tc.tile_pool(name="inblk", bufs=2) as pin,
            tc.tile_pool(name="work", bufs=2) as pw,
            tc.tile_pool(name="psum", bufs=1, space="PSUM") as pps,
            tc.tile_pool(name="dram", bufs=1, space="DRAM") as pd,
        ):
            # ---- stage A: global umin/umax ----
            unc_full = pc.tile([P, F], F32)
            nc.sync.dma_start(out=unc_full, in_=unc_pa)
            lin = pc.tile([P, N_TH], F32)
            nc.sync.dma_start(out=lin, in_=lin_d.ap())

            mm = pc.tile([P, 4], F32)
            nc.vector.tensor_reduce(out=mm[:, 2:3], in_=unc_full,
                                    axis=mybir.AxisListType.X, op=mybir.AluOpType.min)
            nc.vector.tensor_reduce(out=mm[:, 1:2], in_=unc_full,
                                    axis=mybir.AxisListType.X, op=mybir.AluOpType.max)
            # negate min so a single max-allreduce handles both
            nc.vector.tensor_scalar(out=mm[:, 0:1], in0=mm[:, 2:3], scalar1=-1.0,
                                    scalar2=None, op0=mybir.AluOpType.mult)
            mmg = pc.tile([P, 2], F32)
            nc.gpsimd.partition_all_reduce(mmg, mm[:, 0:2], channels=P,
                                           reduce_op=bass_isa.ReduceOp.max)
            cin = pd.tile([1, 2], F32)
            cout = pd.tile([1, 2], F32, addr_space="Shared")
            nc.sync.dma_start(out=cin, in_=mmg[0:1, :])
            nc.gpsimd.collective_compute(
                "AllReduce",
                mybir.AluOpType.max,
                replica_groups=[list(range(N_CORES))],
                ins=[cin.opt()],
                outs=[cout.opt()],
            )
            gmm = pc.tile([P, 2], F32)
            bcast_src = bass.AP(tensor=cout.tensor, offset=cout.offset,
                                ap=[[0, P]] + [list(d) for d in cout.ap[1:]])
            nc.sync.dma_start(out=gmm, in_=bcast_src)
            # th = lin * (umax - umin) + umin, all in fp32 exactly as jax
            umin_s = pc.tile([P, 2], F32)
            nc.vector.tensor_scalar(out=umin_s[:, 0:1], in0=gmm[:, 0:1], scalar1=-1.0,
                                    scalar2=None, op0=mybir.AluOpType.mult)
            nc.vector.tensor_scalar(out=umin_s[:, 1:2], in0=gmm[:, 1:2],
                                    scalar1=gmm[:, 0:1], scalar2=None,
                                    op0=mybir.AluOpType.add)
            th = pc.tile([P, N_TH], F32)
            nc.vector.tensor_scalar(out=th, in0=lin, scalar1=umin_s[:, 1:2],
                                    scalar2=umin_s[:, 0:1], op0=mybir.AluOpType.mult,
                                    op1=mybir.AluOpType.add)

            psum_t = pps.tile([P, N_TH * G], F32)

            for b in range(N_BLK):
                lo2 = b * 2 * FB
                lo1 = b * FB
                probs_b = pin.tile([P, 2 * FB], F32, tag="probs")
                nc.sync.dma_start(out=probs_b, in_=probs_pa[:, lo2:lo2 + 2 * FB])
                lab_b = pin.tile([P, 2 * FB], I32, tag="lab")
                nc.sync.dma_start(out=lab_b, in_=lab_pa[:, lo2:lo2 + 2 * FB])

                p1v = probs_b[:, 1::2]
                p0v = probs_b[:, 0::2]

                stat = pw.tile([P, GB, 8, G], BF16, tag="stat")
                maskt = pw.tile([P, GB, N_TH, G], BF16, tag="mask")
                labf = pw.tile([P, FB], BF16, tag="labf")
                pred = pw.tile([P, FB], BF16, tag="pred")

                def qcol(q):
                    return stat[:, :, q, :]

                lv = labf.rearrange("p (g j) -> p g j", j=G)
                pv = pred.rearrange("p (g j) -> p g j", j=G)
                uv = unc_full[:, lo1:lo1 + FB].rearrange("p (g j) -> p g j", j=G)
                p1g = probs_b.rearrange("p (g j c) -> p g (j c)", g=GB, c=2)[:, :, 1::2]

                # per-sample quantities -> stationary columns
                nc.vector.tensor_copy(labf, lab_b[:, 0::2])
                nc.vector.tensor_tensor(out=pred, in0=p1v, in1=p0v,
                                        op=mybir.AluOpType.is_gt)
                nc.vector.tensor_tensor(out=qcol(1), in0=pv, in1=lv,
                                        op=mybir.AluOpType.is_equal)
                nc.gpsimd.memset(qcol(0), 1.0)
                nc.vector.tensor_copy(qcol(2), p1g)
                nc.scalar.activation(out=qcol(3), in_=uv,
                                     func=mybir.ActivationFunctionType.Tanh)
                nc.vector.tensor_tensor(out=qcol(4), in0=qcol(1), in1=qcol(3),
                                        op=mybir.AluOpType.mult)
                nc.vector.tensor_tensor(out=qcol(5), in0=qcol(2), in1=qcol(3),
                                        op=mybir.AluOpType.mult)
                nc.vector.tensor_tensor(out=qcol(6), in0=qcol(1), in1=qcol(2),
                                        op=mybir.AluOpType.mult)
                nc.vector.tensor_tensor(out=qcol(7), in0=qcol(6), in1=qcol(3),
                                        op=mybir.AluOpType.mult)

                # threshold masks -> moving columns
                nc.gpsimd.memset(maskt[:, :, 0, :], 1.0)
                for k, eng in enumerate(MASK_ENG):
                    dst = maskt[:, :, 1 + k, :]
                    thk = th[:, k:k + 1]
                    if eng == 'v':
                        nc.vector.tensor_scalar(out=dst, in0=uv, scalar1=thk,
                                                scalar2=None, op0=mybir.AluOpType.is_le)
                    else:
                        nc.scalar.activation(out=dst, in_=uv,
                                             func=mybir.ActivationFunctionType.Sign,
                                             bias=thk, scale=-1.0)

                for g in range(GB):
                    gg = b * GB + g
                    nc.tensor.matmul(
                        out=psum_t,
                        lhsT=stat[:, g, :, :],
                        rhs=maskt[:, g, :, :],
                        start=(gg == 0),
                        stop=(gg == N_BLK * GB - 1),
                    )

            out_sb = pc.tile([P, N_TH * G], F32)
            nc.vector.tensor_copy(out_sb, psum_t)
            nc.sync.dma_start(out=out_d.ap(), in_=out_sb)
    nc.compile()
    return nc


def kernel(probs, labels, unc):
    global LAST_RESULTS
    LAST_RESULTS = []
    probs = np.ascontiguousarray(np.asarray(probs, dtype=np.float32))
    unc = np.ascontiguousarray(np.asarray(unc, dtype=np.float32))
    labels = np.asarray(labels)
    if labels.dtype == np.int64:
        lab32 = labels.view(np.int32).reshape(-1, 2)
    else:
        lab32 = np.stack(
            [labels.astype(np.int32), np.zeros_like(labels, dtype=np.int32)], axis=1)

    cores = list(range(N_CORES))
    th_lin = np.linspace(0.0, 1.0, N_TH, dtype=np.float32)

    if FUSED:
        if "fused" not in _CACHE:
            _CACHE["fused"] = _build_fused()
        lin_in = np.ascontiguousarray(np.tile(th_lin[None, :], (P, 1)))
        in2 = []
        for c in cores:
            sl = slice(c * NS, (c + 1) * NS)
            in2.append({
                "probs": np.ascontiguousarray(probs[sl]),
                "lab": np.ascontiguousarray(lab32[sl]),
                "unc": np.ascontiguousarray(unc[sl]),
                "lin": lin_in,
            })
        r2 = run_bass_kernel_spmd(_CACHE["fused"], in2, core_ids=cores, trace=TRACE)
        LAST_RESULTS.append(("main", r2))
    else:
        # ---- launch 1: per-core min/max of unc, host all-reduce ----
        if "mm" not in _CACHE:
            _CACHE["mm"] = _build_minmax()
        in1 = [{"unc": np.ascontiguousarray(unc[c * NS:(c + 1) * NS])} for c in cores]
        r1 = run_bass_kernel_spmd(_CACHE["mm"], in1, core_ids=cores, trace=TRACE)
        LAST_RESULTS.append(("minmax", r1))
        mm = np.stack([r1.results[c]["mm"] for c in cores])
        umin = np.float32(mm[:, :, 0:4].min())
        umax = np.float32(mm[:, :, 4:8].max())

        # thresholds in fp32, exactly as jax computes them
        unc_th = (umin + th_lin * (umax - umin)).astype(np.float32)
        th_in = np.ascontiguousarray(np.tile(unc_th[None, :], (P, 1)))

        # ---- launch 2: main kernel ----
        if "main" not in _CACHE:
            _CACHE["main"] = _build_main()
        in2 = []
        for c in cores:
            sl = slice(c * NS, (c + 1) * NS)
            in2.append({
                "probs": np.ascontiguousarray(probs[sl]),
                "lab": np.ascontiguousarray(lab32[sl]),
                "unc": np.ascontiguousarray(unc[sl]),
                "th": th_in,
            })
        r2 = run_bass_kernel_spmd(_CACHE["main"], in2, core_ids=cores, trace=TRACE)
        LAST_RESULTS.append(("main", r2))

    # ---- host combine (float64) ----
    # psum[m, n] with m = q*G + jw, n = kk*G + jw'; diagonal jw == jw' blocks
    # hold the real sums.
    S_raw = np.zeros((8, N_TH), np.float64)
    for c in cores:
        o = r2.results[c]["out"].astype(np.float64).reshape(P, N_TH, G)
        for q in range(8):
            for jw in range(G):
                S_raw[q, :] += o[q * G + jw, :, jw]
    T = S_raw[:, 0]  # unmasked totals per stationary column

    S_le = np.empty((8, N_TH), np.float64)
    for k, eng in enumerate(MASK_ENG):
        col = S_raw[:, 1 + k]
        S_le[:, k] = (col + T) / 2.0 if eng == 'a' else col
    S_le[:, 20] = T  # k=20 threshold includes every sample

    S0, S1, S2, S3, S4, S5, S6, S7 = S_le
    T3, T4, T5, T7 = T[3], T[4], T[5], T[7]
    Sh = S0 - S1 - S2 + S6        # sum_cert (1-m)(1-p1)
    Sht = S3 - S4 - S5 + S7       # sum_cert (1-m)(1-p1) t
    Tht = T3 - T4 - T5 + T7

    n_ac = S6 - S7
    n_au = T7 - S7
    n_ic = Sh - Sht
    n_iu = Tht - Sht

    avu = (n_ac + n_iu) / (n_ac + n_au + n_ic + n_iu + EPS)
    th64 = th_lin.astype(np.float64)
    auc = np.sum(0.5 * (avu[1:] + avu[:-1]) * (th64[1:] - th64[:-1]))
    loss = -BETA * np.log(auc + EPS)
    return (np.float32(loss), np.float32(auc))
